# revision 8
# baseline (speedup 1.0000x reference)
"""Trainium2 Bass kernel for nn_DiffEmbedding1234.

Reference computation (per batch b):
    xt      = x[b].T                                  # [T, C]
    x_diff  = diff(xt) with leading zero row          # [T, C]
    x_emb   = x_diff @ W_ve.T + b_ve                  # [T, D]
    x_sm    = (ewma_fwd(x_emb) + ewma_bwd(x_emb))/2   # [T, D]
    out     = x_sm @ W_lin.T + b_lin                  # [T, D]

Every stage is linear in x, so the whole network collapses to
    out[b] = F @ (x[b].T @ W_comb) + b_out
where
    F      = C_ewma @ D_diff   (T x T, banded: entries decay as 0.9^|lag|)
    W_comb = (W_lin @ W_ve).T  # [C, D]
    b_out  = W_lin @ b_ve + b_lin   (EWMA of a constant is the constant,
                                     so b_ve passes through the smoother)

F is truncated to a block-tridiagonal matrix of 128x128 blocks (dropped
entries are <= 0.9^128 ~ 1.4e-6 relative; validated ~6e-7 max rel err
against the reference end to end).  Only 7 distinct blocks exist
(interior blocks are Toeplitz-equal); they are deduped host-side.

Sharding: data-parallel over batch B=32 -> 8 cores x 4 batches.  The
EWMA/filter runs along T which stays fully local; the small matrices are
replicated.  Per core, per 128-row output chunk g = (b, k):
    u^T[c,i]  = sum_{s in k-1,k,k+1} (x^T block s).T @ F^T block   (PE)
    out[i,e]  = u^T.T @ W_comb   (PE)  -> + bias (DVE) -> SBUF -> DMA

Raw Bass (no Tile): this walrus build allows only ONE sync-wait per
instruction, which Tile's semaphore assignment violates; with explicit
per-engine streams every dependency is a standalone wait_ge instruction
and monotone per-engine counters subsume older deps for free.
Output DMAs are batched to 1 MiB and alternate between the two HWDGE
rings (SP and ACT) so the ring FIFO is not the bottleneck.
"""

import os
import sys

for _p in ("/opt/trn_rl_repo",):
    if os.path.isdir(_p) and _p not in sys.path:
        sys.path.append(_p)

import numpy as np

ALPHA = 0.1
B, C, T, D = 32, 32, 2048, 512
L = 128
NCH = T // L          # 16 chunks of 128 along T
NCORES = 8
BPC = B // NCORES     # batches per core
NUP = 4               # up_ps PSUM slots
NOP = 3               # op_ps PSUM slots
NU = 4                # u_sb slots
GRP = 4               # chunks per output DMA group (1 MiB)
NOB = 2               # big output SBUF slots


def _build_filter_blocks():
    """Block-tridiagonal float32 blocks of F = C_ewma @ D_diff (float64 math).

    Returns (fts, table): fts is [128, n_uniq*128] with
    fts[j, m*128 + i] = block_m[i, j] (ready as the moving operand of
    u^T[c,i] = sum_j xT[j,c] * F[i,j]); table[(k, src)] = m.
    """
    i = np.arange(T)
    lag = i[:, None] - i[None, :]
    dec = np.where(lag >= 0, 0.9 ** np.clip(lag, 0, None), 0.0)
    A = ALPHA * dec
    A[:, 0] = 0.9 ** i.astype(np.float64)   # x[0] = y[0] boundary
    Bm = A[::-1, ::-1].copy()               # backward EWMA
    Cm = 0.5 * (A + Bm)
    Dm = np.zeros((T, T))
    Dm[i[1:], i[1:]] = 1.0
    Dm[i[1:], i[1:] - 1] = -1.0
    F = (Cm @ Dm).astype(np.float32)

    uniq: dict[bytes, int] = {}
    blocks: list[np.ndarray] = []
    table: dict[tuple[int, int], int] = {}
    for k in range(NCH):
        for src in (k - 1, k, k + 1):
            if src < 0 or src >= NCH:
                continue
            blk = F[k * L:(k + 1) * L, src * L:(src + 1) * L]
            key = blk.tobytes()
            if key not in uniq:
                uniq[key] = len(blocks)
                blocks.append(blk)
            table[(k, src)] = uniq[key]
    fts = np.concatenate([b.T for b in blocks], axis=1)  # [128, n_uniq*128]
    return np.ascontiguousarray(fts, dtype=np.float32), table


_PROGRAM_CACHE: dict = {}


def _build_program(n_uniq: int, table, repeats: int = 1):
    key = (n_uniq, repeats)
    if key in _PROGRAM_CACHE:
        return _PROGRAM_CACHE[key]

    import concourse.bass as bass
    import concourse.mybir as mybir

    f32 = mybir.dt.float32
    ts = bass.ts

    nc = bass.Bass("TRN2")
    xp = nc.dram_tensor("xp", [BPC, 128, NCH * C], f32, kind="ExternalInput")
    fts = nc.dram_tensor("fts", [128, n_uniq * 128], f32, kind="ExternalInput")
    wcomb = nc.dram_tensor("wcomb", [C, D], f32, kind="ExternalInput")
    bias = nc.dram_tensor("bias", [128, D], f32, kind="ExternalInput")
    y = nc.dram_tensor("y", [BPC, T, D], f32, kind="ExternalOutput")

    ft_sb = nc.alloc_sbuf_tensor("ft_sb", [128, n_uniq * 128], f32)
    wc_sb = nc.alloc_sbuf_tensor("wc_sb", [C, D], f32)
    bi_sb = nc.alloc_sbuf_tensor("bi_sb", [128, D], f32)
    xb_sb = [nc.alloc_sbuf_tensor(f"xb{i}", [128, NCH * C], f32) for i in range(2)]
    u_sb = [nc.alloc_sbuf_tensor(f"u{i}", [C, L], f32) for i in range(NU)]
    o_sb = [nc.alloc_sbuf_tensor(f"o{i}", [128, GRP * D], f32) for i in range(NOB)]
    up_ps = [nc.alloc_psum_tensor(f"up{i}", [C, L], f32) for i in range(NUP)]
    op_ps = [nc.alloc_psum_tensor(f"op{i}", [128, D], f32) for i in range(NOP)]

    R = repeats
    G = BPC * NCH            # chunks per repeat
    GT = G * R               # total chunks
    NJ = GT // GRP           # total output DMA groups (1 MiB each)
    NBAT = BPC * R           # total batch iterations

    # helpers over the global chunk index g
    def bk(g):
        return (g % G) // NCH, (g % G) % NCH

    # PE sem values: per chunk, event 1 = scan stop; the op matmul of the
    # PREVIOUS chunk follows (software pipelining).  Replay emission order
    # to get exact counter values.
    scan_done = [0] * GT
    op_done = [0] * GT
    pe = 0
    for g in range(GT):
        pe += 1
        scan_done[g] = pe
        if g >= 1:
            pe += 1
            op_done[g - 1] = pe
    pe += 1
    op_done[GT - 1] = pe

    # output group j covers chunks [4j, 4j+4), ring j%2 (0=SP, 1=ACT).
    # ring_pos(j) = 1-based position of group j on its ring.
    def ring_pos(j):
        return j // 2 + 1

    n_sp_groups = len([j for j in range(NJ) if j % 2 == 0])
    n_act_groups = NJ - n_sp_groups

    with (
        nc.semaphore("s_const") as s_const,
        nc.semaphore("s_xb0") as s_xb0,
        nc.semaphore("s_xb1") as s_xb1,
        nc.semaphore("s_osp") as s_osp,
        nc.semaphore("s_oact") as s_oact,
        nc.semaphore("s_pe") as s_pe,
        nc.semaphore("s_act") as s_act,
        nc.semaphore("s_dve") as s_dve,
    ):
        s_xb = [s_xb0, s_xb1]
        def issue_group_dma(eng, sem, j):
            b, k = bk(GRP * j)
            k4 = k // GRP
            eng.wait_ge(s_dve, GRP * j + GRP)      # all 4 adds of group j
            src = o_sb[j % NOB][:].rearrange("p (kk e) -> p kk e", e=D)
            dst = y[b, k4 * GRP * L:(k4 + 1) * GRP * L, :].rearrange(
                "(kk p) e -> p kk e", p=128
            )
            eng.dma_start(dst, src).then_inc(sem, 16)

        with nc.Block() as block:

            @block.sync
            def _(sync):
                sync.dma_start(ft_sb[:], fts[:]).then_inc(s_const, 16)
                sync.dma_start(wc_sb[:], wcomb[:]).then_inc(s_const, 16)
                sync.dma_start(bi_sb[:], bias[:]).then_inc(s_const, 16)
                for bi in range(NBAT):
                    if bi >= 2:
                        # even groups of batch bi-2 FIRST: the xb-slot wait
                        # below needs batch bi-2's scans, which transitively
                        # need these DMAs (o_sb slot recycling)
                        for j in range(4 * (bi - 2), 4 * (bi - 1)):
                            if j % 2 == 0:
                                issue_group_dma(sync, s_osp, j)
                        # xb slot free once batch bi-2's scan matmuls done
                        sync.wait_ge(s_pe, scan_done[(bi - 2) * NCH + NCH - 1])
                    sync.dma_start(
                        xb_sb[bi % 2][:], xp[bi % BPC]
                    ).then_inc(s_xb[bi % 2], 16)
                for j in range(max(0, 4 * (NBAT - 2)), NJ):
                    if j % 2 == 0:
                        issue_group_dma(sync, s_osp, j)
                sync.wait_ge(s_osp, 16 * n_sp_groups)

            @block.tensor
            def _(tensor):
                for g in range(GT):
                    b, k = bk(g)
                    bi = g // NCH
                    if k == 0:
                        if bi == 0:
                            tensor.wait_ge(s_const, 48)
                        tensor.wait_ge(s_xb[bi % 2], 16 * (bi // 2 + 1))
                    if g >= NUP:
                        tensor.wait_ge(s_act, g - NUP + 1)
                    srcs = [s for s in (k - 1, k, k + 1) if 0 <= s < NCH]
                    up = up_ps[g % NUP]
                    xb = xb_sb[bi % 2]
                    for n, s in enumerate(srcs):
                        mm = nc.tensor.matmul(
                            up[:],
                            xb[:, ts(s, C)],
                            ft_sb[:, ts(table[(k, s)], 128)],
                            start=(n == 0),
                            stop=(n == len(srcs) - 1),
                        )
                    mm.then_inc(s_pe, 1)
                    if g >= 1:
                        p = g - 1
                        tensor.wait_ge(s_act, p + 1)
                        if p >= NOP:
                            tensor.wait_ge(s_dve, p - NOP + 1)
                        nc.tensor.matmul(
                            op_ps[p % NOP][:], u_sb[p % NU][:], wc_sb[:],
                            start=True, stop=True,
                        ).then_inc(s_pe, 1)
                p = GT - 1
                tensor.wait_ge(s_act, p + 1)
                if p >= NOP:
                    tensor.wait_ge(s_dve, p - NOP + 1)
                nc.tensor.matmul(
                    op_ps[p % NOP][:], u_sb[p % NU][:], wc_sb[:],
                    start=True, stop=True,
                ).then_inc(s_pe, 1)

            @block.scalar
            def _(scalar):
                for g in range(GT):
                    scalar.wait_ge(s_pe, scan_done[g])
                    nc.scalar.copy(
                        u_sb[g % NU][:], up_ps[g % NUP][:]
                    ).then_inc(s_act, 1)
                    # odd output groups ride the ACT HWDGE ring; issue
                    # group j = g//GRP - 1 at the start of the next group
                    if g % GRP == 0 and g >= GRP:
                        j = g // GRP - 1
                        if j % 2 == 1:
                            issue_group_dma(scalar, s_oact, j)
                if (NJ - 1) % 2 == 1:
                    issue_group_dma(scalar, s_oact, NJ - 1)
                if n_act_groups:
                    scalar.wait_ge(s_oact, 16 * n_act_groups)

            @block.vector
            def _(vector):
                vector.wait_ge(s_const, 48)       # bias loaded
                for g in range(GT):
                    j = g // GRP
                    kk = g % GRP
                    if kk == 0 and j >= NOB:
                        jj = j - NOB               # same ring as j
                        sem = s_osp if jj % 2 == 0 else s_oact
                        vector.wait_ge(sem, 16 * ring_pos(jj))
                    vector.wait_ge(s_pe, op_done[g])
                    nc.vector.tensor_add(
                        o_sb[j % NOB][:, ts(kk, D)], op_ps[g % NOP][:],
                        bi_sb[:],
                    ).then_inc(s_dve, 1)

    _PROGRAM_CACHE[key] = nc
    return nc


def _prep_inputs(x, W_ve, b_ve, W_lin, b_lin):
    fts, table = _build_filter_blocks()
    n_uniq = fts.shape[1] // 128
    W_comb = (W_lin.astype(np.float64) @ W_ve.astype(np.float64)).T
    b_out = W_lin.astype(np.float64) @ b_ve.astype(np.float64) + b_lin.astype(np.float64)
    # xp[b, p, k*C + c] = x[b, c, k*128 + p]
    xp = (
        x.transpose(0, 2, 1)
        .reshape(B, NCH, 128, C)
        .transpose(0, 2, 1, 3)
        .reshape(B, 128, NCH * C)
    )
    common = {
        "fts": fts,
        "wcomb": np.ascontiguousarray(W_comb, dtype=np.float32),
        "bias": np.ascontiguousarray(
            np.broadcast_to(b_out.astype(np.float32), (128, D))
        ),
    }
    in_maps = [
        {"xp": np.ascontiguousarray(xp[c * BPC:(c + 1) * BPC]), **common}
        for c in range(NCORES)
    ]
    return in_maps, n_uniq, table


def _run(in_maps, n_uniq, table, repeats: int = 1):
    from concourse.bass_utils import run_bass_kernel_spmd

    nc = _build_program(n_uniq, table, repeats=repeats)
    res = run_bass_kernel_spmd(nc, in_maps, list(range(NCORES)))
    return res


def kernel(x, W_ve, b_ve, W_lin, b_lin):
    in_maps, n_uniq, table = _prep_inputs(x, W_ve, b_ve, W_lin, b_lin)
    res = _run(in_maps, n_uniq, table)
    out = np.concatenate([res.results[c]["y"] for c in range(NCORES)], axis=0)
    return out.astype(np.float32, copy=False)


# revision 10
# speedup vs baseline: 1.9616x; 1.9616x over previous
"""Trainium2 Bass kernel for nn_DiffEmbedding1234.

Reference computation (per batch b):
    xt      = x[b].T                                  # [T, C]
    x_diff  = diff(xt) with leading zero row          # [T, C]
    x_emb   = x_diff @ W_ve.T + b_ve                  # [T, D]
    x_sm    = (ewma_fwd(x_emb) + ewma_bwd(x_emb))/2   # [T, D]
    out     = x_sm @ W_lin.T + b_lin                  # [T, D]

Every stage is linear in x, so the whole network collapses to
    out[b] = F @ (x[b].T @ W_comb) + b_out
where
    F      = C_ewma @ D_diff   (T x T, banded: entries decay as 0.9^|lag|)
    W_comb = (W_lin @ W_ve).T  # [C, D]
    b_out  = W_lin @ b_ve + b_lin   (EWMA of a constant is the constant,
                                     so b_ve passes through the smoother)

F's entries decay as 0.9^|lag|, so only near-diagonal blocks matter
(~1e-6 relative truncation, validated end to end vs the reference).

Sharding: data-parallel over batch B=32 -> 8 cores x 4 batches.  The
filter runs along T which stays fully local; small matrices replicated.

Per-core dataflow (all 4 local batches fused into one 128-wide axis
c' = 4*32 channels):
    u^T[c', i-bank] = sum_s (x^T block s).T @ F^T[s-block, bank]   # PE
        - banks of 512 t-outputs, j-window of 5-6 128-blocks,
          full-bank N=512 accumulation in one PSUM bank
    out[t, e] (per batch, chunk) = u_b^T.T @ W_comb                # PE
        - stationary u slice at partition base 32*b (row-tiled)
    + bias via DVE add [128, 2048] -> SBUF -> 1 MiB DMA per group

Raw Bass (no Tile): this walrus build allows only ONE sync-wait per
instruction, which Tile's semaphore assignment violates; with explicit
per-engine streams every dependency is a standalone wait_ge and
monotone per-engine counters subsume older deps.  Instruction count is
kept minimal (~200/core): large fused ops, coarse-grained semaphores.
"""

import os
import sys

for _p in ("/opt/trn_rl_repo",):
    if os.path.isdir(_p) and _p not in sys.path:
        sys.path.append(_p)

import numpy as np

ALPHA = 0.1
B, C, T, D = 32, 32, 2048, 512
L = 128
NCH = T // L          # 16 chunks of 128 along T
NBK = 4               # banks of 4 chunks (512 t) per batch
NCORES = 8
BPC = B // NCORES     # batches per core
CP = BPC * C          # fused channel axis c' = (b, c) = 128


def _build_filter_banks():
    """F^T slices for the banked scan.

    For output bank m (512 t-values) the contraction runs over j-blocks
    s in [4m-1, 4m+4] (one block of history each side of the bank).
    Returns (fts, bank_terms):
      fts [128, n_uniq*512] with the deduped F^T[s-block, bank-range]
      slices; bank_terms[m] = list of (s, slice_index).
    """
    i = np.arange(T)
    lag = i[:, None] - i[None, :]
    dec = np.where(lag >= 0, 0.9 ** np.clip(lag, 0, None), 0.0)
    A = ALPHA * dec
    A[:, 0] = 0.9 ** i.astype(np.float64)   # x[0] = y[0] boundary
    Bm = A[::-1, ::-1].copy()               # backward EWMA
    Cm = 0.5 * (A + Bm)
    Dm = np.zeros((T, T))
    Dm[i[1:], i[1:]] = 1.0
    Dm[i[1:], i[1:] - 1] = -1.0
    FT = (Cm @ Dm).T.astype(np.float32)     # FT[j, i]

    uniq: dict[bytes, int] = {}
    slices: list[np.ndarray] = []
    bank_terms: dict[int, list[tuple[int, int]]] = {}
    for m in range(NBK):
        terms = []
        for s in range(4 * m - 1, 4 * m + 5):
            if s < 0 or s >= NCH:
                continue
            blk = FT[s * L:(s + 1) * L, m * 4 * L:(m + 1) * 4 * L]  # [128,512]
            key = blk.tobytes()
            if key not in uniq:
                uniq[key] = len(slices)
                slices.append(blk)
            terms.append((s, uniq[key]))
        bank_terms[m] = terms
    fts = np.concatenate(slices, axis=1)    # [128, n_uniq*512]
    return np.ascontiguousarray(fts, dtype=np.float32), bank_terms


_PROGRAM_CACHE: dict = {}


def _build_program(n_uniq: int, bank_terms, repeats: int = 1):
    key = (n_uniq, repeats)
    if key in _PROGRAM_CACHE:
        return _PROGRAM_CACHE[key]

    import concourse.bass as bass
    import concourse.mybir as mybir

    f32 = mybir.dt.float32
    ts = bass.ts

    nc = bass.Bass("TRN2")
    xq = nc.dram_tensor("xq", [128, NCH * CP], f32, kind="ExternalInput")
    fts = nc.dram_tensor("fts", [128, n_uniq * 4 * L], f32, kind="ExternalInput")
    wcr = nc.dram_tensor("wcr", [CP, D], f32, kind="ExternalInput")
    bias = nc.dram_tensor("bias", [128, 4 * D], f32, kind="ExternalInput")
    y = nc.dram_tensor("y", [BPC, T, D], f32, kind="ExternalOutput")

    xq_sb = nc.alloc_sbuf_tensor("xq_sb", [128, NCH * CP], f32)
    ft_sb = nc.alloc_sbuf_tensor("ft_sb", [128, n_uniq * 4 * L], f32)
    wc_sb = nc.alloc_sbuf_tensor("wc_sb", [CP, D], f32)
    bi_sb = nc.alloc_sbuf_tensor("bi_sb", [128, 4 * D], f32)
    u_sb = [nc.alloc_sbuf_tensor(f"u{i}", [128, 4 * L], f32) for i in range(2)]
    o_sb = [nc.alloc_sbuf_tensor(f"o{i}", [128, 4 * D], f32) for i in range(2)]
    up_ps = [nc.alloc_psum_tensor(f"up{i}", [128, 4 * L], f32) for i in range(2)]
    op_ps = nc.alloc_psum_tensor("op", [128, 4 * D], f32)

    R = repeats
    # per repeat: 4 scan banks; per bank: 4 batches' op groups; group index
    # gidx = rep*16 + m*4 + b, writes y[b, m*512:(m+1)*512, :]

    # replay PE counter
    scan_done = {}
    op_done = {}
    pe = 0
    for r in range(R):
        for m in range(NBK):
            pe += 1
            scan_done[(r, m)] = pe
            for b in range(BPC):
                pe += 1
                op_done[(r, m, b)] = pe

    with (
        nc.semaphore("s_const") as s_const,
        nc.semaphore("s_x") as s_x,
        nc.semaphore("s_o0") as s_o0,
        nc.semaphore("s_o1") as s_o1,
        nc.semaphore("s_pe") as s_pe,
        nc.semaphore("s_act") as s_act,
        nc.semaphore("s_dve") as s_dve,
    ):
        s_o = [s_o0, s_o1]

        with nc.Block() as block:

            @block.sync
            def _(sync):
                sync.dma_start(ft_sb[:], fts[:]).then_inc(s_const, 16)
                sync.dma_start(wc_sb[:], wcr[:]).then_inc(s_const, 16)
                sync.dma_start(bi_sb[:], bias[:]).then_inc(s_const, 16)
                for r in range(R):
                    if r > 0:
                        # xq slot reusable once the previous repeat's scans
                        # are done
                        sync.wait_ge(s_pe, scan_done[(r - 1, NBK - 1)])
                    sync.dma_start(xq_sb[:], xq[:]).then_inc(s_x, 16)
                    for m in range(NBK):
                        for b in range(BPC):
                            gidx = r * 16 + m * 4 + b
                            sync.wait_ge(s_dve, gidx + 1)  # add done
                            sync.dma_start(
                                y[b, m * 4 * L:(m + 1) * 4 * L, :].rearrange(
                                    "(kk p) e -> p kk e", p=L
                                ),
                                o_sb[gidx % 2][:].rearrange(
                                    "p (kk e) -> p kk e", e=D
                                ),
                            ).then_inc(s_o[gidx % 2], 16)
                # drain: all output DMAs landed
                sync.wait_ge(s_o0, 16 * (R * 8))
                sync.wait_ge(s_o1, 16 * (R * 8))

            @block.tensor
            def _(tensor):
                tensor.wait_ge(s_const, 48)
                for r in range(R):
                    tensor.wait_ge(s_x, 16 * (r + 1))
                    for m in range(NBK):
                        bank_idx = r * NBK + m
                        if bank_idx >= 2:
                            # up_ps slot free once its ACT copy (2 banks
                            # ago) is done
                            tensor.wait_ge(s_act, bank_idx - 1)
                        terms = bank_terms[m]
                        up = up_ps[bank_idx % 2]
                        for n, (s, sl) in enumerate(terms):
                            mm = nc.tensor.matmul(
                                up[:],
                                xq_sb[:, ts(s, CP)],
                                ft_sb[:, ts(sl, 4 * L)],
                                start=(n == 0),
                                stop=(n == len(terms) - 1),
                            )
                        mm.then_inc(s_pe, 1)
                        # ops for this bank need its u copy
                        tensor.wait_ge(s_act, bank_idx + 1)
                        u = u_sb[bank_idx % 2]
                        for b in range(BPC):
                            gidx = r * 16 + m * 4 + b
                            if gidx >= 1:
                                # op_ps free once the previous group's add
                                # is done
                                tensor.wait_ge(s_dve, gidx)
                            for kk in range(4):
                                mm = nc.tensor.matmul(
                                    op_ps[:, ts(kk, D)],
                                    u[b * C:(b + 1) * C, ts(kk, L)],
                                    wc_sb[b * C:(b + 1) * C, :],
                                    start=True, stop=True,
                                    tile_position=(b * C, 0),
                                )
                            mm.then_inc(s_pe, 1)

            @block.scalar
            def _(scalar):
                for r in range(R):
                    for m in range(NBK):
                        bank_idx = r * NBK + m
                        # scan done; also subsumes u_sb slot release (ops of
                        # bank_idx-2 precede scan(bank_idx) in PE order)
                        scalar.wait_ge(s_pe, scan_done[(r, m)])
                        nc.scalar.copy(
                            u_sb[bank_idx % 2][:], up_ps[bank_idx % 2][:]
                        ).then_inc(s_act, 1)

            @block.vector
            def _(vector):
                vector.wait_ge(s_const, 48)
                for r in range(R):
                    for m in range(NBK):
                        for b in range(BPC):
                            gidx = r * 16 + m * 4 + b
                            if gidx >= 2:
                                # o_sb slot free once its DMA (2 groups ago)
                                # completed
                                vector.wait_ge(
                                    s_o[gidx % 2], 16 * (gidx // 2)
                                )
                            vector.wait_ge(s_pe, op_done[(r, m, b)])
                            nc.vector.tensor_add(
                                o_sb[gidx % 2][:], op_ps[:], bi_sb[:]
                            ).then_inc(s_dve, 1)

    _PROGRAM_CACHE[key] = nc
    return nc


def _prep_inputs(x, W_ve, b_ve, W_lin, b_lin):
    fts, bank_terms = _build_filter_banks()
    n_uniq = fts.shape[1] // (4 * L)
    W_comb = (W_lin.astype(np.float64) @ W_ve.astype(np.float64)).T  # [C, D]
    b_out = W_lin.astype(np.float64) @ b_ve.astype(np.float64) + b_lin.astype(np.float64)
    # xq[p, k*CP + b*C + c] = x[b, c, k*128 + p]
    xq_all = (
        x.reshape(B, C, NCH, L)
        .transpose(3, 2, 0, 1)           # [p, k, b, c]  (b within full B)
        .reshape(L, NCH, B, C)
    )
    wcr = np.tile(W_comb.astype(np.float32), (BPC, 1))          # [128, D]
    bias4 = np.tile(b_out.astype(np.float32), 4)                 # [4*D]
    common = {
        "fts": fts,
        "wcr": np.ascontiguousarray(wcr),
        "bias": np.ascontiguousarray(
            np.broadcast_to(bias4.astype(np.float32), (128, 4 * D))
        ),
    }
    in_maps = []
    for cc in range(NCORES):
        xq = xq_all[:, :, cc * BPC:(cc + 1) * BPC, :].reshape(L, NCH * CP)
        in_maps.append({"xq": np.ascontiguousarray(xq), **common})
    return in_maps, n_uniq, bank_terms


def _run(in_maps, n_uniq, bank_terms, repeats: int = 1):
    from concourse.bass_utils import run_bass_kernel_spmd

    nc = _build_program(n_uniq, bank_terms, repeats=repeats)
    res = run_bass_kernel_spmd(nc, in_maps, list(range(NCORES)))
    return res


def kernel(x, W_ve, b_ve, W_lin, b_lin):
    in_maps, n_uniq, bank_terms = _prep_inputs(x, W_ve, b_ve, W_lin, b_lin)
    res = _run(in_maps, n_uniq, bank_terms)
    out = np.concatenate([res.results[c]["y"] for c in range(NCORES)], axis=0)
    return out.astype(np.float32, copy=False)
